# revision 25
# baseline (speedup 1.0000x reference)
"""Trainium2 Bass kernel for nn_DotProductAttention_31679678775913.

Math
----
The reference augments scalar sequences via a Linear(1, 64):
  q_i = query_i * W + b   (same for key / value)
  scores[i,j] = <q_i, k_j> = a*query_i*key_j + c*(query_i + key_j) + d
      with a = W.W, c = W.b, d = b.b
  weights = softmax_j(scores)  -- the per-row constants c*query_i + d cancel,
      so weights[i,j] = softmax_j(alpha_i * key_j) with alpha_i = a*query_i + c.
  attended[i,:] = (sum_j weights[i,j] * value_j) * W + b   (softmax sums to 1)

So per batch element the work collapses to a rank-1 logit matrix:
  m_i   = max(alpha_i*kmax, alpha_i*kmin)            (exact row max)
  E     = exp(alpha_i*k_j - m_i)   [2048, 2048]      (ACT, one pass, row-sums Z
                                                      accumulated in the same op)
  wt    = E * (1/Z_i)                                (ACT/DVE alternating)
  u_i   = (sum_j E_ij v_j) / Z_i                     (DVE fused mul+reduce)
  att   = u_i * W + b
Dominant cost is streaming the 16.8 MB weights output per core to HBM
(~47us at ~358 GB/s/core); compute is balanced just under that.

Sharding: batch dim B=8, one batch element per NeuronCore (8 cores).
"""

import numpy as np

import concourse.bass as bass
import concourse.tile as tile
from concourse import bacc, mybir
from concourse.bass_utils import run_bass_kernel_spmd

B, L, N = 8, 2048, 64
PT = 128          # partitions per tile
NT = L // PT      # 16 row tiles
F32 = mybir.dt.float32
AF = mybir.ActivationFunctionType
ALU = mybir.AluOpType

# Fraction of normalize passes that run on ACT (rest on DVE); tune for balance.
ACT_NORM_EVERY = 2  # t % ACT_NORM_EVERY == 0 -> ACT does the normalize


def build_nc() -> bass.Bass:
    # Bacc (not raw Bass): its compile() pass splits multi-sem waits into
    # event semaphores (TRN2 allows one wait per instruction) and inserts
    # GPSIMD library / ACT table loads.
    #
    # Host-side prep (part of input sharding in kernel()): alpha = a*q + c,
    # negm = -rowmax of the rank-1 logits, wb = W||b packed. All O(L) scalar
    # prep; every O(L^2) op runs on device.
    nc = bacc.Bacc("TRN2", debug=False)

    al_d = nc.declare_dram_parameter("alpha", [L], F32, isOutput=False)
    nm_d = nc.declare_dram_parameter("negm", [L], F32, isOutput=False)
    k_d = nc.declare_dram_parameter("k", [L], F32, isOutput=False)
    v_d = nc.declare_dram_parameter("v", [L], F32, isOutput=False)
    wb_d = nc.declare_dram_parameter("wb", [2 * N], F32, isOutput=False)
    wt_d = nc.declare_dram_parameter("weights", [L, L], F32, isOutput=True)
    at_d = nc.declare_dram_parameter("attended", [L, N], F32, isOutput=True)

    with tile.TileContext(nc) as tc:
        with (
            tc.tile_pool(name="singles", bufs=1) as singles,
            tc.tile_pool(name="epool", bufs=4) as epool,
            tc.tile_pool(name="wpool", bufs=6) as wpool,
            tc.tile_pool(name="spool", bufs=3) as spool,
            tc.tile_pool(name="zpool", bufs=6) as zpool,
        ):
            # ---------------- loads ----------------
            # Row mapping: partition p of tile t holds sequence row i = 16*p + t,
            # so alpha/negm loads are plain contiguous reshapes (fast DMA
            # descriptors) while each weights-tile store still writes one
            # contiguous 8KB row per partition.
            # Broadcast key/value across all 128 partitions straight from DRAM,
            # both on the SWDGE ring and strictly k before v: the first exp
            # waits only on k_bc, and v_bc must not steal SDMA slots from it.
            # The small loads ride the separate HWDGE (sync) ring in parallel,
            # which then stays dedicated to the weights store stream.
            k_bc = singles.tile([PT, L], F32)
            nc.gpsimd.dma_start(
                out=k_bc[:, 0 : L // 2],
                in_=k_d.ap()[0 : L // 2].partition_broadcast(PT),
            )
            nc.sync.dma_start(
                out=k_bc[:, L // 2 : L],
                in_=k_d.ap()[L // 2 : L].partition_broadcast(PT),
            )
            v_bc = singles.tile([PT, L], F32)
            nc.gpsimd.dma_start(out=v_bc[:], in_=v_d.ap().partition_broadcast(PT))

            alpha = singles.tile([PT, NT], F32)
            nc.sync.dma_start(
                out=alpha[:], in_=al_d.ap().rearrange("(p t) -> p t", p=PT)
            )
            negm = singles.tile([PT, NT], F32)
            nc.sync.dma_start(
                out=negm[:], in_=nm_d.ap().rearrange("(p t) -> p t", p=PT)
            )
            wb_bc = singles.tile([PT, 2 * N], F32)
            nc.sync.dma_start(out=wb_bc[:], in_=wb_d.ap().partition_broadcast(PT))
            w_bc = wb_bc[:, 0:N]
            b_bc = wb_bc[:, N : 2 * N]

            # Tiny warm-up activation so the exp table load happens during the
            # DMA prologue instead of right before the first real exp.
            warm = singles.tile([1, 8], F32)
            nc.vector.memset(warm[:], 0.0)
            nc.scalar.activation(warm[:], warm[:], AF.Exp)

            # attended rows accumulate here, one 64-wide chunk per tile
            att_all = singles.tile([PT, NT * N], F32)

            # ---------------- main loop over 16 row tiles ----------------
            for t in range(NT):
                e_t = epool.tile([PT, L], F32)
                zs = zpool.tile([PT, 1], F32, tag="zs")
                # E = exp(alpha_p * k - m_p), Z = row sum -- one ACT pass
                nc.scalar.activation(
                    e_t[:],
                    k_bc[:],
                    AF.Exp,
                    bias=negm[:, t : t + 1],
                    scale=alpha[:, t : t + 1],
                    accum_out=zs[:],
                )
                rz = zpool.tile([PT, 1], F32, tag="rz")
                nc.vector.reciprocal(rz[:], zs[:])

                wt_t = wpool.tile([PT, L], F32)
                if t % 2 == 0 or t == 15:
                    nc.scalar.mul(wt_t[:], e_t[:], rz[:])
                else:
                    nc.vector.tensor_scalar(
                        wt_t[:], e_t[:], rz[:], None, op0=ALU.mult
                    )

                # u = sum_j wt_ij * v_j via one fused DVE pass:
                # out = (E * 1/Z) * v (scratch, discarded), accum_out = row sum
                scr = spool.tile([PT, L], F32)
                u = zpool.tile([PT, 1], F32, tag="u")
                nc.vector.scalar_tensor_tensor(
                    scr[:],
                    e_t[:],
                    rz[:],
                    v_bc[:],
                    op0=ALU.mult,
                    op1=ALU.mult,
                    accum_out=u[:],
                )

                # attended chunk: u * W + b
                nc.vector.scalar_tensor_tensor(
                    att_all[:, t * N : (t + 1) * N],
                    w_bc,
                    u[:],
                    b_bc,
                    op0=ALU.mult,
                    op1=ALU.add,
                )

                nc.sync.dma_start(
                    out=wt_d.ap().rearrange("(p u) j -> p u j", u=NT)[:, t, :],
                    in_=wt_t[:],
                )

                # stream the attended rows out in 4-tile chunks so the final
                # chunk doesn't sit in the kernel tail
                if t % 4 == 3:
                    c0 = t - 3
                    nc.sync.dma_start(
                        out=at_d.ap()
                        .rearrange("(p u) n -> p u n", u=NT)[:, c0 : t + 1, :],
                        in_=att_all[:].rearrange("p (t n) -> p t n", n=N)[
                            :, c0 : t + 1, :
                        ],
                    )

    nc.finalize()
    return nc


_CACHE: dict = {}


def _get_nc() -> bass.Bass:
    if "nc" not in _CACHE:
        _CACHE["nc"] = build_nc()
    return _CACHE["nc"]


# test.py can flip this to get a traced (profiled) run; the result object of the
# last hardware run is stashed in LAST_RESULTS.
TRACE = False
TRACE_DIR = None
LAST_RESULTS = None


def kernel(query, key, value, W, b):
    global LAST_RESULTS
    query = np.ascontiguousarray(np.asarray(query, np.float32))
    key = np.ascontiguousarray(np.asarray(key, np.float32))
    value = np.ascontiguousarray(np.asarray(value, np.float32))
    W_flat = np.ascontiguousarray(np.asarray(W, np.float32).reshape(-1))
    b_flat = np.ascontiguousarray(np.asarray(b, np.float32).reshape(-1))
    assert query.shape == (B, L) and key.shape == (B, L) and value.shape == (B, L)
    assert W_flat.shape == (N,) and b_flat.shape == (N,)

    # Host-side scalar prep (all O(L); the O(L^2) attention runs on device):
    # scores collapse to alpha_i * k_j with alpha = (W.W) q + (W.b); softmax
    # rowmax is max(alpha*kmax, alpha*kmin) since logits are monotone in k.
    a = np.float32(np.sum(W_flat.astype(np.float32) * W_flat, dtype=np.float32))
    c = np.float32(np.sum(W_flat * b_flat, dtype=np.float32))
    alpha = (a * query + c).astype(np.float32)  # [B, L]
    kmax = key.max(axis=1, keepdims=True)
    kmin = key.min(axis=1, keepdims=True)
    negm = -np.maximum(alpha * kmax, alpha * kmin).astype(np.float32)  # [B, L]
    wb = np.concatenate([W_flat, b_flat]).astype(np.float32)

    nc = _get_nc()
    in_maps = [
        {
            "alpha": alpha[c_],
            "negm": negm[c_],
            "k": key[c_],
            "v": value[c_],
            "wb": wb,
        }
        for c_ in range(B)
    ]
    res = run_bass_kernel_spmd(
        nc, in_maps, list(range(B)), trace=TRACE, tmpdir=TRACE_DIR
    )
    LAST_RESULTS = res
    weights = np.stack([res.results[c]["weights"] for c in range(B)])
    attended = np.stack([res.results[c]["attended"] for c in range(B)])
    return attended, weights


# revision 26
# speedup vs baseline: 1.0485x; 1.0485x over previous
"""Trainium2 Bass kernel for nn_DotProductAttention_31679678775913.

Math
----
The reference augments scalar sequences via a Linear(1, 64):
  q_i = query_i * W + b   (same for key / value)
  scores[i,j] = <q_i, k_j> = a*query_i*key_j + c*(query_i + key_j) + d
      with a = W.W, c = W.b, d = b.b
  weights = softmax_j(scores)  -- the per-row constants c*query_i + d cancel,
      so weights[i,j] = softmax_j(alpha_i * key_j) with alpha_i = a*query_i + c.
  attended[i,:] = (sum_j weights[i,j] * value_j) * W + b   (softmax sums to 1)

So per batch element the work collapses to a rank-1 logit matrix:
  m_i   = max(alpha_i*kmax, alpha_i*kmin)            (exact row max)
  E     = exp(alpha_i*k_j - m_i)   [2048, 2048]      (ACT, one pass, row-sums Z
                                                      accumulated in the same op)
  wt    = E * (1/Z_i)                                (ACT/DVE alternating)
  u_i   = (sum_j E_ij v_j) / Z_i                     (DVE fused mul+reduce)
  att   = u_i * W + b
Dominant cost is streaming the 16.8 MB weights output per core to HBM
(~47us at ~358 GB/s/core); compute is balanced just under that.

Sharding: batch dim B=8, one batch element per NeuronCore (8 cores).
"""

import numpy as np

import concourse.bass as bass
import concourse.tile as tile
from concourse import bacc, mybir
from concourse.bass_utils import run_bass_kernel_spmd

B, L, N = 8, 2048, 64
PT = 128          # partitions per tile
NT = L // PT      # 16 row tiles
F32 = mybir.dt.float32
AF = mybir.ActivationFunctionType
ALU = mybir.AluOpType

# Fraction of normalize passes that run on ACT (rest on DVE); tune for balance.
ACT_NORM_EVERY = 2  # t % ACT_NORM_EVERY == 0 -> ACT does the normalize


def build_nc() -> bass.Bass:
    # Bacc (not raw Bass): its compile() pass splits multi-sem waits into
    # event semaphores (TRN2 allows one wait per instruction) and inserts
    # GPSIMD library / ACT table loads.
    #
    # Host-side prep (part of input sharding in kernel()): alpha = a*q + c,
    # negm = -rowmax of the rank-1 logits, wb = W||b packed. All O(L) scalar
    # prep; every O(L^2) op runs on device.
    nc = bacc.Bacc("TRN2", debug=False)

    al_d = nc.declare_dram_parameter("alpha", [L], F32, isOutput=False)
    nm_d = nc.declare_dram_parameter("negm", [L], F32, isOutput=False)
    k_d = nc.declare_dram_parameter("k", [L], F32, isOutput=False)
    v_d = nc.declare_dram_parameter("v", [L], F32, isOutput=False)
    wb_d = nc.declare_dram_parameter("wb", [2 * N], F32, isOutput=False)
    wt_d = nc.declare_dram_parameter("weights", [L, L], F32, isOutput=True)
    at_d = nc.declare_dram_parameter("attended", [L, N], F32, isOutput=True)

    with tile.TileContext(nc) as tc:
        with (
            tc.tile_pool(name="singles", bufs=1) as singles,
            tc.tile_pool(name="epool", bufs=4) as epool,
            tc.tile_pool(name="wpool", bufs=6) as wpool,
            tc.tile_pool(name="spool", bufs=3) as spool,
            tc.tile_pool(name="zpool", bufs=6) as zpool,
        ):
            # ---------------- loads ----------------
            # Row mapping: partition p of tile t holds sequence row i = 16*p + t,
            # so alpha/negm loads are plain contiguous reshapes (fast DMA
            # descriptors) while each weights-tile store still writes one
            # contiguous 8KB row per partition.
            # Broadcast key/value across all 128 partitions straight from DRAM,
            # both on the SWDGE ring and strictly k before v: the first exp
            # waits only on k_bc, and v_bc must not steal SDMA slots from it.
            # The small loads ride the separate HWDGE (sync) ring in parallel,
            # which then stays dedicated to the weights store stream.
            k_bc = singles.tile([PT, L], F32)
            nc.gpsimd.dma_start(out=k_bc[:], in_=k_d.ap().partition_broadcast(PT))
            v_bc = singles.tile([PT, L], F32)
            nc.gpsimd.dma_start(out=v_bc[:], in_=v_d.ap().partition_broadcast(PT))

            alpha = singles.tile([PT, NT], F32)
            nc.sync.dma_start(
                out=alpha[:], in_=al_d.ap().rearrange("(p t) -> p t", p=PT)
            )
            negm = singles.tile([PT, NT], F32)
            nc.sync.dma_start(
                out=negm[:], in_=nm_d.ap().rearrange("(p t) -> p t", p=PT)
            )
            wb_bc = singles.tile([PT, 2 * N], F32)
            nc.sync.dma_start(out=wb_bc[:], in_=wb_d.ap().partition_broadcast(PT))
            w_bc = wb_bc[:, 0:N]
            b_bc = wb_bc[:, N : 2 * N]

            # Tiny warm-up activation so the exp table load happens during the
            # DMA prologue instead of right before the first real exp.
            warm = singles.tile([1, 8], F32)
            nc.vector.memset(warm[:], 0.0)
            nc.scalar.activation(warm[:], warm[:], AF.Exp)

            # attended rows accumulate here, one 64-wide chunk per tile
            att_all = singles.tile([PT, NT * N], F32)

            # ---------------- main loop over 16 row tiles ----------------
            for t in range(NT):
                e_t = epool.tile([PT, L], F32)
                zs = zpool.tile([PT, 1], F32, tag="zs")
                # E = exp(alpha_p * k - m_p), Z = row sum -- one ACT pass
                nc.scalar.activation(
                    e_t[:],
                    k_bc[:],
                    AF.Exp,
                    bias=negm[:, t : t + 1],
                    scale=alpha[:, t : t + 1],
                    accum_out=zs[:],
                )
                rz = zpool.tile([PT, 1], F32, tag="rz")
                nc.vector.reciprocal(rz[:], zs[:])

                wt_t = wpool.tile([PT, L], F32)
                if t % 2 == 0 or t == 15:
                    nc.scalar.mul(wt_t[:], e_t[:], rz[:])
                else:
                    nc.vector.tensor_scalar(
                        wt_t[:], e_t[:], rz[:], None, op0=ALU.mult
                    )

                # u = sum_j wt_ij * v_j via one fused DVE pass:
                # out = (E * 1/Z) * v (scratch, discarded), accum_out = row sum
                scr = spool.tile([PT, L], F32)
                u = zpool.tile([PT, 1], F32, tag="u")
                nc.vector.scalar_tensor_tensor(
                    scr[:],
                    e_t[:],
                    rz[:],
                    v_bc[:],
                    op0=ALU.mult,
                    op1=ALU.mult,
                    accum_out=u[:],
                )

                # attended chunk: u * W + b
                nc.vector.scalar_tensor_tensor(
                    att_all[:, t * N : (t + 1) * N],
                    w_bc,
                    u[:],
                    b_bc,
                    op0=ALU.mult,
                    op1=ALU.add,
                )

                nc.sync.dma_start(
                    out=wt_d.ap().rearrange("(p u) j -> p u j", u=NT)[:, t, :],
                    in_=wt_t[:],
                )

                # stream the attended rows out in 4-tile chunks so the final
                # chunk doesn't sit in the kernel tail
                if t % 4 == 3:
                    c0 = t - 3
                    nc.sync.dma_start(
                        out=at_d.ap()
                        .rearrange("(p u) n -> p u n", u=NT)[:, c0 : t + 1, :],
                        in_=att_all[:].rearrange("p (t n) -> p t n", n=N)[
                            :, c0 : t + 1, :
                        ],
                    )

    nc.finalize()
    return nc


_CACHE: dict = {}


def _get_nc() -> bass.Bass:
    if "nc" not in _CACHE:
        _CACHE["nc"] = build_nc()
    return _CACHE["nc"]


# test.py can flip this to get a traced (profiled) run; the result object of the
# last hardware run is stashed in LAST_RESULTS.
TRACE = False
TRACE_DIR = None
LAST_RESULTS = None


def kernel(query, key, value, W, b):
    global LAST_RESULTS
    query = np.ascontiguousarray(np.asarray(query, np.float32))
    key = np.ascontiguousarray(np.asarray(key, np.float32))
    value = np.ascontiguousarray(np.asarray(value, np.float32))
    W_flat = np.ascontiguousarray(np.asarray(W, np.float32).reshape(-1))
    b_flat = np.ascontiguousarray(np.asarray(b, np.float32).reshape(-1))
    assert query.shape == (B, L) and key.shape == (B, L) and value.shape == (B, L)
    assert W_flat.shape == (N,) and b_flat.shape == (N,)

    # Host-side scalar prep (all O(L); the O(L^2) attention runs on device):
    # scores collapse to alpha_i * k_j with alpha = (W.W) q + (W.b); softmax
    # rowmax is max(alpha*kmax, alpha*kmin) since logits are monotone in k.
    a = np.float32(np.sum(W_flat.astype(np.float32) * W_flat, dtype=np.float32))
    c = np.float32(np.sum(W_flat * b_flat, dtype=np.float32))
    alpha = (a * query + c).astype(np.float32)  # [B, L]
    kmax = key.max(axis=1, keepdims=True)
    kmin = key.min(axis=1, keepdims=True)
    negm = -np.maximum(alpha * kmax, alpha * kmin).astype(np.float32)  # [B, L]
    wb = np.concatenate([W_flat, b_flat]).astype(np.float32)

    nc = _get_nc()
    in_maps = [
        {
            "alpha": alpha[c_],
            "negm": negm[c_],
            "k": key[c_],
            "v": value[c_],
            "wb": wb,
        }
        for c_ in range(B)
    ]
    res = run_bass_kernel_spmd(
        nc, in_maps, list(range(B)), trace=TRACE, tmpdir=TRACE_DIR
    )
    LAST_RESULTS = res
    weights = np.stack([res.results[c]["weights"] for c in range(B)])
    attended = np.stack([res.results[c]["attended"] for c in range(B)])
    return attended, weights
